# revision 8
# baseline (speedup 1.0000x reference)
"""Distributed Trainium2 Bass kernel for a dense-transformer attention block.

Sharding (8 NeuronCores): core cid = 4*b + g
  - b = batch index (B=2), g = kv-head group (N_KV_HEADS=4)
  - Each core: LN1(x[b]) -> its 4 query heads + its 1 kv head (column
    parallel wq/wk/wv), RoPE, causal GQA attention with pre-ictal bias,
    AllGather of per-group attention outputs (groups [0..3], [4..7]),
    LN2, column-parallel wo -> output columns [512g:512g+512].
  - Host concatenates the 8 output shards.

Schedule: 4 software-pipelined "columns" (512 seq rows each):
  col g4: LN1 stats/norm of rows -> XBAR DMA transpose into ln1T ->
  K/V/Q projections + RoPE for that seq block -> attention chunk g4
  (+ AllGather) -> LN2+wo for an earlier chunk whose gather landed.
Compute dtype: bf16 matmul operands, f32 PSUM accumulation, f32 softmax/LN.
"""

import math
from contextlib import ExitStack

import numpy as np
import ml_dtypes

import concourse.bass as bass
import concourse.bacc as bacc
import concourse.mybir as mybir
import concourse.tile as tile
from concourse.bass_utils import run_bass_kernel_spmd

# Problem constants (hardcoded per spec nn_Attention_36120674959366)
B = 2
S = 2048
DIM = 2048
N_HEADS = 16
N_KV_HEADS = 4
HEAD_DIM = 128
NH_LOC = N_HEADS // N_KV_HEADS  # 4 q-heads per core
DQ_LOC = NH_LOC * HEAD_DIM      # 512
PRE_ICTAL_WINDOW = 10
PRE_ICTAL_BIAS = 2.0
LN_EPS = 1e-5
NEG_INF = -1e9

SQD = math.sqrt(HEAD_DIM)           # 11.3137085
INV_SQD = 1.0 / SQD
BIAS_SCALED = PRE_ICTAL_BIAS * SQD  # 22.627417
NEG_SCALED = NEG_INF * SQD          # -1.13137085e10

NT = S // 128                        # 16 tiles of 128 rows
NC = DIM // 128                      # 16 dim chunks

F32 = mybir.dt.float32
BF16 = mybir.dt.bfloat16

# attention chunks of query tiles: (first_tile, n_tiles)
CHUNKS = [(0, 4), (4, 4), (8, 4), (12, 2), (14, 2)]

_CACHED = {}


def build_nc():
    nc = bacc.Bacc("TRN2", target_bir_lowering=False, debug=False, num_devices=8)

    # ---- kernel I/O (per-core shards; same graph on all 8 cores) ----
    xs = nc.dram_tensor("xs", [S, DIM], BF16, kind="ExternalInput")
    wqT = nc.dram_tensor("wqT", [DIM, DQ_LOC], BF16, kind="ExternalInput")
    wkT = nc.dram_tensor("wkT", [DIM, HEAD_DIM], BF16, kind="ExternalInput")
    wvT = nc.dram_tensor("wvT", [DIM, HEAD_DIM], BF16, kind="ExternalInput")
    woT = nc.dram_tensor("woT", [DIM, DQ_LOC], BF16, kind="ExternalInput")
    qb = nc.dram_tensor("qb", [128, NH_LOC], F32, kind="ExternalInput")
    kb = nc.dram_tensor("kb", [128, 1], F32, kind="ExternalInput")
    vbt = nc.dram_tensor("vbt", [128, HEAD_DIM], BF16, kind="ExternalInput")
    obt = nc.dram_tensor("obt", [128, DQ_LOC], F32, kind="ExternalInput")
    labels = nc.dram_tensor("labels", [S], F32, kind="ExternalInput")
    cosT = nc.dram_tensor("cosT", [HEAD_DIM, S], BF16, kind="ExternalInput")
    sinT = nc.dram_tensor("sinT", [HEAD_DIM, S], BF16, kind="ExternalInput")
    ident = nc.dram_tensor("ident", [128, 128], BF16, kind="ExternalInput")
    pswap = nc.dram_tensor("pswap", [128, 128], BF16, kind="ExternalInput")
    ctri = nc.dram_tensor("ctri", [128, 128], BF16, kind="ExternalInput")
    out = nc.dram_tensor("out", [S, DQ_LOC], F32, kind="ExternalOutput")

    AF = mybir.ActivationFunctionType
    OP = mybir.AluOpType

    with tile.TileContext(nc) as tc, ExitStack() as st:
        pc = st.enter_context(tc.tile_pool(name="const", bufs=1))
        dr = st.enter_context(tc.tile_pool(name="dr", bufs=1, space="DRAM"))
        pw = st.enter_context(tc.tile_pool(name="wts", bufs=1))
        pm = st.enter_context(tc.tile_pool(name="work", bufs=1))
        ps = st.enter_context(tc.tile_pool(name="ps", bufs=1, space="PSUM"))

        # ======== priority DMA: first column's x tiles ========
        xt_tiles = {}

        def load_xt(i):
            xt = pm.tile([128, DIM], BF16, tag="xt", bufs=4, name=f"xt_{i}")
            nc.sync.dma_start(out=xt[:, :], in_=xs[128 * i:128 * i + 128, :])
            xt_tiles[i] = xt

        for i in range(4):
            load_xt(i)

        # ======== constants into SBUF ========
        ident_sb = pc.tile([128, 128], BF16, tag="ident")
        nc.sync.dma_start(out=ident_sb[:, :], in_=ident[:, :])
        pswap_sb = pc.tile([128, 128], BF16, tag="pswap")
        nc.sync.dma_start(out=pswap_sb[:, :], in_=pswap[:, :])
        ctri_sb = pc.tile([128, 128], BF16, tag="ctri")
        nc.sync.dma_start(out=ctri_sb[:, :], in_=ctri[:, :])
        cos_sb = pc.tile([128, S], BF16, tag="cos")
        nc.sync.dma_start(out=cos_sb[:, :], in_=cosT[:, :])
        sin_sb = pc.tile([128, S], BF16, tag="sin")
        nc.sync.dma_start(out=sin_sb[:, :], in_=sinT[:, :])
        qb_sb = pc.tile([128, NH_LOC], F32, tag="qb")
        nc.sync.dma_start(out=qb_sb[:, :], in_=qb[:, :])
        kb_sb = pc.tile([128, 1], F32, tag="kb")
        nc.sync.dma_start(out=kb_sb[:, :], in_=kb[:, :])
        vb_sb = pc.tile([128, HEAD_DIM], BF16, tag="vb")
        nc.sync.dma_start(out=vb_sb[:, :], in_=vbt[:, :])

        # ======== weights (emitted after x prefetch; bf16) ========
        wq_sb = pw.tile([128, NC, DQ_LOC], BF16, tag="wq")
        nc.sync.dma_start(
            out=wq_sb[:, :, :],
            in_=wqT.ap().rearrange("(c p) n -> p c n", p=128))
        wk_sb = pw.tile([128, NC, HEAD_DIM], BF16, tag="wk")
        nc.sync.dma_start(
            out=wk_sb[:, :, :],
            in_=wkT.ap().rearrange("(c p) n -> p c n", p=128))
        wv_sb = pw.tile([128, NC, HEAD_DIM], BF16, tag="wv")
        nc.sync.dma_start(
            out=wv_sb[:, :, :],
            in_=wvT.ap().rearrange("(c p) n -> p c n", p=128))
        wo_sb = pw.tile([128, NC, DQ_LOC], BF16, tag="wo")
        nc.sync.dma_start(
            out=wo_sb[:, :, :],
            in_=woT.ap().rearrange("(c p) n -> p c n", p=128))
        ob_sb = pc.tile([128, DQ_LOC], F32, tag="ob")
        nc.sync.dma_start(out=ob_sb[:, :], in_=obt[:, :])

        eps_sb = pc.tile([128, 1], F32, tag="eps")
        nc.vector.memset(eps_sb[:, :], LN_EPS)
        ones_col = pc.tile([1, 128], BF16, tag="ones_col")
        nc.vector.memset(ones_col[:, :], 1.0)

        # LN statistics tiles (absolute-tile indexed)
        mv_all = pc.tile([128, NT, 2], F32, tag="mv_all")
        s_all = pc.tile([128, NT], F32, tag="s_all")
        rs_all = pc.tile([128, NT], F32, tag="rs_all")
        mv2_all = pc.tile([128, NT, 2], F32, tag="mv2_all")
        s2_all = pc.tile([128, NT], F32, tag="s2_all")
        rs2_all = pc.tile([128, NT], F32, tag="rs2_all")

        # ======== seizure-label cumulative sums -> colv/rb ========
        colv_bf = pc.tile([128, NT], BF16, tag="colv_bf")
        colv = pc.tile([128, NT], F32, tag="colv")
        rb_sb = pc.tile([128, S], BF16, tag="rb_sb")

        pcs = tc.alloc_tile_pool(name="csum", bufs=1)
        lab_sb = pcs.tile([1, S], F32, tag="lab")
        nc.sync.dma_start(out=lab_sb[:, :],
                          in_=labels.ap().rearrange("(o s) -> o s", o=1))

        def emit_csum():
            csrow = pcs.tile([1, S + 12], F32, tag="csrow")
            nc.vector.memset(csrow[:, 0:1], 0.0)
            nc.vector.tensor_tensor_scan(
                out=csrow[:, 1:S + 1],
                data0=lab_sb[:, :],
                data1=lab_sb[:, :],
                initial=0.0,
                op0=OP.add,
                op1=OP.bypass,
            )
            for j in range(11):
                nc.vector.tensor_copy(csrow[:, S + 1 + j:S + 2 + j],
                                      csrow[:, S:S + 1])
            csrow_bf = pcs.tile([1, S + 12], BF16, tag="csrow_bf")
            nc.vector.tensor_copy(csrow_bf[:, :], csrow[:, :])

            # colv[p, kt] = cs[min(128*kt + p + 10, S-1)] via DRAM bounce +
            # XBAR transpose (bf16 exact for small counts)
            csbuf = dr.tile([S + 12], BF16)
            nc.sync.dma_start(out=csbuf.rearrange("(o s) -> o s", o=1),
                              in_=csrow_bf[:, :])
            cs16 = pcs.tile([NT, 128], BF16, tag="cs16")
            nc.sync.dma_start(
                out=cs16[:, :],
                in_=csbuf[11:11 + S].rearrange("(t p) -> t p", p=128),
            )
            nc.sync.dma_start_transpose(colv_bf[:, :], cs16[:, :])
            nc.vector.tensor_copy(colv[:, :], colv_bf[:, :])

            # rb_sb[p, q] = csrow[q] broadcast down partitions (cnt_a source)
            for sgm in range(4):
                rbp = ps.tile([128, 1024], F32, tag="mega", bufs=2,
                              name=f"rbp_{sgm}")
                nc.tensor.matmul(rbp[:, 0:512], lhsT=ones_col[:, :],
                                 rhs=csrow_bf[:, 512 * sgm:512 * sgm + 512],
                                 start=True, stop=True)
                nc.vector.tensor_copy(rb_sb[:, 512 * sgm:512 * sgm + 512],
                                      rbp[:, 0:512])
            pcs.release()

        # ======== persistent QKV tiles ========
        qT = pm.tile([128, NH_LOC, S], BF16, tag="qT")
        kT = pm.tile([128, S], BF16, tag="kT")
        v_aug = pm.tile([128, NT, 132], BF16, tag="v_aug")
        nc.gpsimd.memset(v_aug[:, :, 128:129], 1.0)

        bounce_outs = {}
        afs_tiles = {}

        # ---------------- helpers ----------------
        def emit_ln1_stats(g4):
            for j4 in range(4):
                i = 4 * g4 + j4
                xt = xt_tiles[i]
                st6 = pm.tile([128, 4, 6], F32, tag="st6a", bufs=2)
                for a4 in range(4):
                    nc.vector.bn_stats(st6[:, a4, :],
                                       xt[:, 512 * a4:512 * a4 + 512])
                nc.vector.bn_aggr(mv_all[:, i, :], st6[:, :, :])

        def emit_ln1_sqrt(g4):
            sl = slice(4 * g4, 4 * g4 + 4)
            nc.scalar.activation(s_all[:, sl], mv_all[:, sl, 1:2], AF.Sqrt,
                                 bias=eps_sb[:, :])
            nc.vector.reciprocal(rs_all[:, sl], s_all[:, sl])

        def emit_ln1_norm(g4, ln1T):
            for j4 in range(4):
                i = 4 * g4 + j4
                xt = xt_tiles.pop(i)
                xh = pm.tile([128, DIM], BF16, tag="xh", bufs=2)
                nc.vector.tensor_scalar(
                    out=xh[:, :], in0=xt[:, :],
                    scalar1=mv_all[:, i, 0:1], scalar2=rs_all[:, i:i + 1],
                    op0=OP.subtract, op1=OP.mult)
                nc.sync.dma_start_transpose(
                    ln1T[:, :, 128 * j4:128 * j4 + 128], xh[:, :])

        def rope_sg(dst_sg, w_sb, h, sg, ln1T):
            bias_ap = kb_sb[:, 0:1] if h is None else qb_sb[:, h:h + 1]
            pq = ps.tile([128, 1024], F32, tag="mega", bufs=2,
                         name=f"pq_{h}_{sg}")
            for c in range(NC):
                lhsT = w_sb[:, c, :] if h is None \
                    else w_sb[:, c, 128 * h:128 * h + 128]
                nc.tensor.matmul(pq[:, 0:512], lhsT=lhsT,
                                 rhs=ln1T[:, c, 0:512],
                                 start=(c == 0), stop=(c == NC - 1))
            raw = pm.tile([128, 512], BF16, tag="rope_raw", bufs=2)
            nc.scalar.activation(raw[:, :], pq[:, 0:512], AF.Identity,
                                 bias=bias_ap)
            pw2 = ps.tile([128, 1024], F32, tag="mega", bufs=2,
                          name=f"pw2_{h}_{sg}")
            nc.tensor.matmul(pw2[:, 0:512], lhsT=pswap_sb[:, :],
                             rhs=raw[:, :], start=True, stop=True)
            t1 = pm.tile([128, 512], BF16, tag="rope_t1", bufs=2)
            nc.vector.tensor_mul(t1[:, :], raw[:, :],
                                 cos_sb[:, 512 * sg:512 * sg + 512])
            t2 = pm.tile([128, 512], BF16, tag="rope_t2", bufs=2)
            nc.vector.tensor_mul(t2[:, :], pw2[:, 0:512],
                                 sin_sb[:, 512 * sg:512 * sg + 512])
            nc.vector.tensor_add(dst_sg, t1[:, :], t2[:, :])

        def emit_qkv(g4, ln1T):
            sg = g4
            # K projection + rope for this seq block
            rope_sg(kT[:, 512 * sg:512 * sg + 512], wk_sb, None, sg, ln1T)
            # V projection for the 4 seq tiles of this block
            vps = ps.tile([128, 1024], F32, tag="mega", bufs=2,
                          name=f"vps_{g4}")
            for j4 in range(4):
                i = 4 * g4 + j4
                reg = vps[:, 128 * j4:128 * j4 + 128]
                for c in range(NC):
                    nc.tensor.matmul(
                        reg, lhsT=ln1T[:, c, 128 * j4:128 * j4 + 128],
                        rhs=wv_sb[:, c, :],
                        start=(c == 0), stop=(c == NC - 1),
                        skip_group_check=True)
            for j4 in range(4):
                i = 4 * g4 + j4
                nc.vector.tensor_add(v_aug[:, i, 0:128],
                                     vps[:, 128 * j4:128 * j4 + 128],
                                     vb_sb[:, :])
            # Q projections + rope
            for h in range(NH_LOC):
                rope_sg(qT[:, h, 512 * sg:512 * sg + 512], wq_sb, h, sg, ln1T)

        def build_mg(ci):
            s0, w = CHUNKS[ci]
            kts = list(range(max(0, s0 - 1), s0 + w))
            mg = pm.tile([128, 5, 512], BF16, tag="mg", bufs=2,
                         name=f"mg_{ci}")
            nc.vector.memset(mg[:, :, :], 0.0)
            mg_idx = {}
            for slot, kt in enumerate(kts):
                mg_idx[kt] = slot
                if s0 <= kt < s0 + w:  # diag: qtile t == kt
                    j = kt - s0
                    sl = mg[:, slot, 128 * j:128 * j + 128]
                    nc.vector.tensor_scalar(
                        out=sl, in0=rb_sb[:, 128 * kt:128 * kt + 128],
                        scalar1=colv[:, kt:kt + 1], scalar2=BIAS_SCALED,
                        op0=OP.is_lt, op1=OP.mult,
                    )
                    nc.vector.tensor_add(sl, sl, ctri_sb[:, :])
                tprev = kt + 1
                if s0 <= tprev < s0 + w:  # prev: qtile t == kt + 1
                    j = tprev - s0
                    sl = mg[:, slot, 128 * j:128 * j + 128]
                    nc.vector.tensor_scalar(
                        out=sl, in0=rb_sb[:, 128 * tprev:128 * tprev + 128],
                        scalar1=colv[:, kt:kt + 1], scalar2=BIAS_SCALED,
                        op0=OP.is_lt, op1=OP.mult,
                    )
            return mg, mg_idx

        def emit_attention(ci, mid=None):
            s0, w = CHUNKS[ci]
            mg, mg_idx = build_mg(ci)
            attn_c = pm.tile([128, w, DQ_LOC], BF16, tag="attn_c",
                             bufs=2, name=f"attn_c{ci}")
            for h in range(NH_LOC):
                if h == 1 and mid is not None:
                    mid()
                nkt = s0 + w
                pvp = [ps.tile([128, 132], F32, tag="pv_acc", bufs=4,
                               name=f"pv_{h}_{ci}_{jj}")
                       for jj in range(w)]
                kt = 0
                while kt < nkt:
                    mega = ps.tile([128, 1024], F32, tag="mega", bufs=2,
                                   name=f"sc_{h}_{ci}_{kt}")
                    pair = [k2 for k2 in (kt, kt + 1) if k2 < nkt]
                    offs = []
                    for slot, k2 in enumerate(pair):
                        off = 128 * max(0, k2 - s0)
                        offs.append(off)
                        reg = mega[:, 512 * slot + off:
                                   512 * slot + 128 * w]
                        if k2 in mg_idx:
                            nc.tensor.matmul(
                                reg, lhsT=ident_sb[:, :],
                                rhs=mg[:, mg_idx[k2], off:128 * w],
                                start=True, stop=False)
                            nc.tensor.matmul(
                                reg,
                                lhsT=kT[:, 128 * k2:128 * k2 + 128],
                                rhs=qT[:, h, 128 * s0 + off:
                                       128 * (s0 + w)],
                                start=False, stop=True)
                        else:
                            nc.tensor.matmul(
                                reg,
                                lhsT=kT[:, 128 * k2:128 * k2 + 128],
                                rhs=qT[:, h, 128 * s0 + off:
                                       128 * (s0 + w)],
                                start=True, stop=True)
                    pt = pm.tile([128, 1024], BF16, tag="pt_sm", bufs=2)
                    if len(pair) == 2 and offs[0] == 0 and offs[1] == 0:
                        if w == 4:
                            nc.scalar.activation(pt[:, :], mega[:, :],
                                                 AF.Exp, scale=INV_SQD)
                        else:
                            for slot in range(2):
                                nc.scalar.activation(
                                    pt[:, 512 * slot:512 * slot + 128 * w],
                                    mega[:, 512 * slot:512 * slot + 128 * w],
                                    AF.Exp, scale=INV_SQD)
                    else:
                        for slot, k2 in enumerate(pair):
                            off = offs[slot]
                            nc.scalar.activation(
                                pt[:, 512 * slot + off:512 * slot + 128 * w],
                                mega[:, 512 * slot + off:512 * slot + 128 * w],
                                AF.Exp, scale=INV_SQD)
                    for slot, k2 in enumerate(pair):
                        for j in range(max(0, k2 - s0), w):
                            nc.tensor.matmul(
                                pvp[j][:, 0:129],
                                lhsT=pt[:, 512 * slot + 128 * j:
                                        512 * slot + 128 * j + 128],
                                rhs=v_aug[:, k2, 0:129],
                                start=(k2 == 0), stop=(k2 == s0 + j),
                                skip_group_check=True)
                    kt += len(pair)
                for j in range(w):
                    rcp = pm.tile([128, 1], F32, tag="rcp", bufs=4,
                                  name=f"rcp_{h}_{ci}_{j}")
                    nc.vector.reciprocal(rcp[:, :], pvp[j][:, 128:129])
                    nc.vector.tensor_single_scalar(
                        out=attn_c[:, j, 128 * h:128 * h + 128],
                        in_=pvp[j][:, 0:128],
                        scalar=rcp[:, :],
                        op=OP.mult)

            bounce_in = dr.tile([128 * w, DQ_LOC], BF16, name=f"bin{ci}")
            nc.sync.dma_start(
                out=bounce_in.rearrange("(t p) n -> p t n", p=128),
                in_=attn_c[:, :, :])
            bounce_out = dr.tile([4, 128 * w, DQ_LOC], BF16, name=f"bout{ci}")
            nc.gpsimd.collective_compute(
                "AllGather",
                mybir.AluOpType.bypass,
                replica_groups=[[0, 1, 2, 3], [4, 5, 6, 7]],
                ins=[bounce_in[:, :].opt()],
                outs=[bounce_out[:, :, :].opt()],
            )
            bounce_outs[ci] = bounce_out

        def emit_ln2_stats(ci):
            s0, w = CHUNKS[ci]
            for j in range(w):
                t = s0 + j
                afs = pm.tile([128, 4, 512], BF16, tag="afs", bufs=4,
                              name=f"afs_{t}")
                nc.scalar.dma_start(
                    out=afs[:, :, :],
                    in_=bounce_outs[ci][:, 128 * j:128 * j + 128, :]
                        .rearrange("g p n -> p g n"))
                afs_tiles[t] = afs
                st6b = pm.tile([128, 4, 6], F32, tag="st6b", bufs=2)
                for a4 in range(4):
                    nc.vector.bn_stats(st6b[:, a4, :], afs[:, a4, :])
                nc.vector.bn_aggr(mv2_all[:, t, :], st6b[:, :, :])
            sl = slice(s0, s0 + w)
            nc.scalar.activation(s2_all[:, sl], mv2_all[:, sl, 1:2], AF.Sqrt,
                                 bias=eps_sb[:, :])
            nc.vector.reciprocal(rs2_all[:, sl], s2_all[:, sl])

        def emit_ln2_mm(ci):
            s0, w = CHUNKS[ci]
            for j in range(w):
                t = s0 + j
                afs = afs_tiles.pop(t)
                xh2 = pm.tile([128, DIM], BF16, tag="xh2", bufs=2,
                              name=f"xh2_{t}")
                nc.vector.tensor_scalar(
                    out=xh2[:, :],
                    in0=afs.rearrange("p g n -> p (g n)"),
                    scalar1=mv2_all[:, t, 0:1], scalar2=rs2_all[:, t:t + 1],
                    op0=OP.subtract, op1=OP.mult)
                ln2T = pm.tile([128, NC, 128], BF16, tag="ln2T", bufs=2,
                               name=f"ln2T_{t}")
                nc.sync.dma_start_transpose(ln2T[:, :, :], xh2[:, :])
                po = ps.tile([128, 1024], F32, tag="mega", bufs=2,
                             name=f"po_{t}")
                for c in range(NC):
                    nc.tensor.matmul(
                        po[:, 0:512],
                        lhsT=ln2T[:, c, :],
                        rhs=wo_sb[:, c, :],
                        start=(c == 0), stop=(c == NC - 1))
                osb = pm.tile([128, DQ_LOC], F32, tag="osb", bufs=2,
                              name=f"osb_{t}")
                nc.vector.tensor_add(osb[:, :], po[:, 0:512], ob_sb[:, :])
                nc.sync.dma_start(out=out[128 * t:128 * t + 128, :],
                                   in_=osb[:, :])

        # ================= pipelined columns =================
        # col g4: [prefetch x(g4+1)] LN1(g4) QKV(g4) [ln2 stats]
        #         ATTN(chunk) [+AG] [ln2 mm]
        for g4 in range(4):
            if g4 < 3:
                for i in range(4 * g4 + 4, 4 * g4 + 8):
                    load_xt(i)
            emit_ln1_stats(g4)
            emit_ln1_sqrt(g4)
            ln1T = pm.tile([128, NC, 512], BF16, tag="ln1T", bufs=1,
                           name=f"ln1T_{g4}")
            emit_ln1_norm(g4, ln1T)
            emit_qkv(g4, ln1T)
            if g4 == 0:
                emit_csum()
            mid = (lambda ci=g4 - 1: emit_ln2_stats(ci)) if g4 >= 1 else None
            emit_attention(g4, mid=mid)
            if g4 >= 1:
                emit_ln2_mm(g4 - 1)
            if g4 == 3:
                emit_attention(4, mid=lambda: emit_ln2_stats(3))
                emit_ln2_mm(3)
        # tail
        emit_ln2_stats(4)
        emit_ln2_mm(4)

    nc.compile()
    return nc


def _prep_inputs(x, freqs_cis, seizure_labels, wq, wk, wv, wo,
                 ln1_w, ln1_b, ln2_w, ln2_b):
    bf16 = ml_dtypes.bfloat16
    cos = np.asarray(freqs_cis[..., 0], dtype=np.float32)  # [S, 64]
    sin = np.asarray(freqs_cis[..., 1], dtype=np.float32)
    cosT = np.ascontiguousarray(np.repeat(cos.T, 2, axis=0), dtype=bf16)
    sgn = np.where(np.arange(HEAD_DIM) % 2 == 0, -1.0, 1.0).astype(np.float32)
    sinT = np.ascontiguousarray(np.repeat(sin.T, 2, axis=0) * sgn[:, None],
                                dtype=bf16)
    ident = np.eye(128, dtype=bf16)
    psw = np.zeros((128, 128), dtype=np.float32)
    idx = np.arange(128)
    psw[idx ^ 1, idx] = 1.0  # out[m, s] = sum_k psw[k, m] * in[k, s] = in[m^1, s]
    psw = psw.astype(bf16)
    kk = np.arange(128)[:, None]
    qq = np.arange(128)[None, :]
    ctri_np = np.where(qq >= kk, 0.0, NEG_SCALED).astype(bf16)

    # fold LN affine weights into the projection weights (host-side
    # preprocessing, standard inference-time weight folding):
    #   ln(x)@W.T = xhat@(W*w).T + b@W.T
    w1 = np.asarray(ln1_w, np.float64)
    b1 = np.asarray(ln1_b, np.float64)
    w2 = np.asarray(ln2_w, np.float64)
    b2 = np.asarray(ln2_b, np.float64)
    in_maps = []
    for cid in range(8):
        b, g = divmod(cid, 4)
        wq_s = np.asarray(wq[DQ_LOC * g:DQ_LOC * (g + 1), :], np.float64)
        wk_s = np.asarray(wk[HEAD_DIM * g:HEAD_DIM * (g + 1), :], np.float64)
        wv_s = np.asarray(wv[HEAD_DIM * g:HEAD_DIM * (g + 1), :], np.float64)
        wo_s = np.asarray(wo[DQ_LOC * g:DQ_LOC * (g + 1), :], np.float64)
        qb_v = (b1 @ wq_s.T).astype(np.float32)         # [512]
        kb_v = (b1 @ wk_s.T).astype(np.float32)         # [128]
        vb_v = (b1 @ wv_s.T).astype(np.float32)         # [128]
        ob_v = (b2 @ wo_s.T).astype(np.float32)         # [512]
        in_maps.append({
            "xs": np.ascontiguousarray(x[b], dtype=bf16),
            "wqT": np.ascontiguousarray((wq_s * w1).T, dtype=bf16),
            "wkT": np.ascontiguousarray((wk_s * w1).T, dtype=bf16),
            "wvT": np.ascontiguousarray((wv_s * w1).T, dtype=bf16),
            "woT": np.ascontiguousarray((wo_s * w2).T, dtype=bf16),
            "qb": np.ascontiguousarray(
                qb_v.reshape(NH_LOC, 128).T, dtype=np.float32),
            "kb": np.ascontiguousarray(kb_v.reshape(128, 1), dtype=np.float32),
            "vbt": np.ascontiguousarray(np.tile(vb_v, (128, 1)), dtype=bf16),
            "obt": np.ascontiguousarray(np.tile(ob_v, (128, 1)),
                                        dtype=np.float32),
            "labels": np.ascontiguousarray(seizure_labels[b], dtype=np.float32),
            "cosT": cosT, "sinT": sinT,
            "ident": ident, "pswap": psw, "ctri": ctri_np,
        })
    return in_maps


def run(inputs, trace=False, trace_cores=None):
    x = np.asarray(inputs["x"])
    mask = np.asarray(inputs["mask"])
    # this kernel specializes the additive mask to the causal prefill mask
    causal = np.where(np.tril(np.ones((S, S), dtype=bool)), 0.0, NEG_INF
                      ).astype(np.float32)
    if not np.array_equal(mask, causal):
        raise NotImplementedError("kernel specialized for causal prefill mask")

    in_maps = _prep_inputs(
        x, np.asarray(inputs["freqs_cis"]), np.asarray(inputs["seizure_labels"]),
        np.asarray(inputs["wq"]), np.asarray(inputs["wk"]),
        np.asarray(inputs["wv"]), np.asarray(inputs["wo"]),
        np.asarray(inputs["ln1_w"]), np.asarray(inputs["ln1_b"]),
        np.asarray(inputs["ln2_w"]), np.asarray(inputs["ln2_b"]))

    if "nc" not in _CACHED:
        _CACHED["nc"] = build_nc()
    nc = _CACHED["nc"]

    kw = {}
    if trace:
        kw = dict(trace=True,
                  trace_cores=trace_cores if trace_cores is not None else [0])
    res = run_bass_kernel_spmd(nc, in_maps, core_ids=list(range(8)), **kw)

    shards = [res.results[cid]["out"] for cid in range(8)]
    full = np.empty((B, S, DIM), dtype=np.float32)
    for cid in range(8):
        b, g = divmod(cid, 4)
        full[b, :, DQ_LOC * g:DQ_LOC * (g + 1)] = shards[cid]
    return full, res


def kernel(**inputs) -> np.ndarray:
    out, _ = run(inputs, trace=False)
    return out
